# revision 1
# baseline (speedup 1.0000x reference)
"""Trainium2 Bass kernel for nn_MultiHeadCrossAttention (B=16, Dq=768, H=12,
hd=64, Nq=1024, Nt=64, Dkv=384) with RoPE on q and k.

Sharding: pure data-parallel over batch, 2 batches per core across 8 cores.
No collectives.

Per-core dataflow (all "T" tensors are channel-major, i.e. transposed):
  qT  = Wq.T @ feat            (PE, f32r, output stays transposed)
  qc  = qT * cos_q * scale     (DVE, fused with PSUM->SBUF move)
  qs  = qT * sin_q * scale     (DVE)
  kT  = Wk.T @ tokensT         (PE)  -> RoPE-combined into kA (=k_rot) and kB
  v   = tokens @ Wv            (PE, natural layout, duplicated across
                                partition halves so both heads of a pair
                                contract in their own array quadrant)
  scoresT = kA.T@qc + kB.T@qs  (PE, PSUM-accumulated: RoPE needs no shuffles
                                on the q side; the half-rotation is folded
                                into the k-side tensors and the table pair)
  E = exp(scoresT)             (ACT, no max-subtraction: |scores| <= ~1.3)
  D = blockdiag_ones.T @ E     (PE, all 12 head denominators into one PSUM tile)
  R = 1/D                      (DVE)
  B = indicator.T @ R          (PE, broadcasts each head's recip row to 64
                                partitions)
  E = E * B                    (DVE, normalize)
  attnT = v.T @ E              (PE)
  out = feat + Wout.T @ attnT + bias   (PE + one fused DVE op)
"""

import os
import sys
from contextlib import ExitStack

import numpy as np

sys.path.insert(0, "/opt/trn_rl_repo")

import concourse.bass as bass  # noqa: E402
import concourse.mybir as mybir  # noqa: E402
import concourse.tile as tile  # noqa: E402
from concourse import bacc  # noqa: E402
from concourse.bass_utils import run_bass_kernel_spmd  # noqa: E402

import ml_dtypes

F32 = mybir.dt.float32
BF16 = mybir.dt.bfloat16
NPBF = ml_dtypes.bfloat16

B, DQ, T, HP, WP = 16, 768, 4, 16, 16
NQ = T * HP * WP            # 1024
NT, DKV = 64, 384
H, HD = 12, 64
SCALE = HD ** -0.5
NCORES = 8
BL = B // NCORES            # batches per core = 2
CHUNK = 512                 # query positions per chunk
NCH = NQ // CHUNK           # chunks per batch = 2
KQ = DQ // 128              # 6 contraction tiles for Dq
KKV = DKV // 128            # 3 contraction tiles for Dkv
NPAIR = H // 2              # 6 head pairs


def _rope_tables(n):
    inv_freq = 1.0 / (10000.0 ** (np.arange(0, HD, 2, dtype=np.float64) / HD))
    freqs = np.arange(n, dtype=np.float64)[:, None] * inv_freq[None, :]
    emb = np.concatenate([freqs, freqs], axis=-1)  # [n, 64]
    return (np.cos(emb).T.astype(np.float32), np.sin(emb).T.astype(np.float32))


def _consts():
    cq, sq = _rope_tables(NQ)          # [64, 1024]
    ck, sk = _rope_tables(NT)          # [64, 64]
    # q tables: scale folded in, duplicated across the two heads of a pair
    cq2 = np.ascontiguousarray(np.tile(cq * SCALE, (2, 1)))       # [128, 1024]
    sq2 = np.ascontiguousarray(np.tile(sq * SCALE, (2, 1)))
    # k tables: duplicated 2 heads (partitions) x 2 batches (columns),
    # tiled KQ times along free so k-RoPE runs as one batched op
    ck2 = np.ascontiguousarray(np.tile(ck, (2, 2 * KQ)))          # [128, 768]
    sk2 = np.ascontiguousarray(np.tile(sk, (2, 2 * KQ)))
    eps = np.where(np.arange(HD) < HD // 2, -1.0, 1.0).astype(np.float32)
    epsv = np.ascontiguousarray(np.tile(eps, 2)[:, None])         # [128, 1]
    ident = np.eye(128, dtype='float32')
    # denominator lhsT: for pair j, col 2j sums partitions 0-63 (even head),
    # col 2j+1 sums partitions 64-127 (odd head)
    dlhs = np.zeros((128, NPAIR, H), np.float32)
    for j in range(NPAIR):
        dlhs[:64, j, 2 * j] = 1.0
        dlhs[64:, j, 2 * j + 1] = 1.0
    # broadcast lhsT: for pair j, row 2j feeds cols 0-63, row 2j+1 cols 64-127
    blhs = np.zeros((H, NPAIR, 128), np.float32)
    for j in range(NPAIR):
        blhs[2 * j, j, :64] = 1.0
        blhs[2 * j + 1, j, 64:] = 1.0
    import ml_dtypes as _md
    bf = _md.bfloat16
    return dict(cq=cq2, sq=sq2, ck=ck2, sk=sk2, epsv=epsv, nepsv=-epsv,
                ident=ident.astype(bf), dlhs=dlhs.astype(bf),
                blhs=blhs.astype(bf))


def _sigma_dma(nc, out_ap, in_ap):
    """out = in with 32-partition halves swapped inside each 64 block.
    On the gpsimd SWDGE ring: tiny transfers, and the sync/scalar rings
    are saturated with the front-of-kernel bulk loads."""
    for dst, src in ((0, 32), (32, 0), (64, 96), (96, 64)):
        nc.gpsimd.dma_start(out=out_ap[dst:dst + 32], in_=in_ap[src:src + 32])


def build(debug=False):
    nc = bacc.Bacc(None, target_bir_lowering=False, debug=debug)
    with tile.TileContext(nc) as tc:
        with tc.tile_pool(name="dram", bufs=1, space="DRAM") as dram:
            def din(name, shape, dt=F32):
                return dram.tile(shape, dt, kind="ExternalInput", name=name,
                                 uniquify=False)

            feat_l = din("feat_l", [BL, 128, KQ, NQ])
            feat_bf = din("feat_bf", [BL, 128, KQ, NQ], BF16)
            tok_l = din("tok_l", [BL * NT, DKV], BF16)
            wq = din("wq", [128, KQ, DQ], BF16)
            wk = din("wk", [128, KKV, DQ], BF16)
            wv = din("wv", [128, KKV, DQ], BF16)
            wout = din("wout", [128, KQ, DQ], BF16)
            bout_t = din("bout_t", [128, KQ])
            cq = din("cq", [128, NQ])
            sq = din("sq", [128, NQ])
            ck = din("ck", [128, KQ * 128])
            sk = din("sk", [128, KQ * 128])
            epsv = din("epsv", [128, 1])
            nepsv = din("nepsv", [128, 1])
            ident = din("ident", [128, 128], BF16)
            dlhs = din("dlhs", [128, NPAIR, H], BF16)
            blhs = din("blhs", [H, NPAIR, 128], BF16)
            out_l = dram.tile([BL, 128, KQ, NQ], F32, kind="ExternalOutput",
                              name="out_l", uniquify=False)

            with ExitStack() as body_ctx:
                global _body_ctx
                _body_ctx = body_ctx
                _body(nc, tc, feat_l, feat_bf, tok_l, wq, wk, wv, wout,
                      bout_t, cq, sq, ck, sk, epsv, nepsv, ident, dlhs, blhs,
                      out_l)
    nc.compile()
    return nc


def _body(nc, tc, feat_l, feat_bf, tok_l, wq, wk, wv, wout, bout_t, cq,
          sq, ck, sk, epsv, nepsv, ident, dlhs, blhs, out_l):
    MULT = mybir.AluOpType.mult
    ADD = mybir.AluOpType.add
    EXP = mybir.ActivationFunctionType.Exp

    ctx = _body_ctx
    consts = ctx.enter_context(tc.tile_pool(name="consts", bufs=1))
    kside = ctx.enter_context(tc.tile_pool(name="kside", bufs=1))
    ktmp = ctx.enter_context(tc.tile_pool(name="ktmp", bufs=1))
    featp = ctx.enter_context(tc.tile_pool(name="featp", bufs=2))
    qp = ctx.enter_context(tc.tile_pool(name="qp", bufs=2))
    ep = ctx.enter_context(tc.tile_pool(name="ep", bufs=2))
    atp = ctx.enter_context(tc.tile_pool(name="atp", bufs=2))
    outp = ctx.enter_context(tc.tile_pool(name="outp", bufs=2))
    rp = ctx.enter_context(tc.tile_pool(name="rp", bufs=2))

    pp = ctx.enter_context(tc.tile_pool(name="pp", bufs=3, space="PSUM"))
    attn = ctx.enter_context(tc.tile_pool(name="attn", bufs=4, space="PSUM"))
    dp = ctx.enter_context(tc.tile_pool(name="dp", bufs=1, space="PSUM"))

    # ---- load constants. Emission order = DGE ring order: the sync ring
    # carries the phase-0/qproj critical path, the scalar ring the bulk.
    tok_sb = consts.tile([128, DKV], BF16)
    nc.sync.dma_start(out=tok_sb, in_=tok_l[:])
    id_sb = consts.tile([128, 128], BF16)
    nc.sync.dma_start(out=id_sb, in_=ident[:])
    wk_sb = consts.tile([128, KKV, DQ], BF16)
    nc.sync.dma_start(out=wk_sb, in_=wk[:])
    wq_sb = consts.tile([128, KQ, DQ], BF16)
    nc.sync.dma_start(out=wq_sb, in_=wq[:])
    wv_sb = consts.tile([128, KKV, DQ], BF16)
    nc.scalar.dma_start(out=wv_sb, in_=wv[:])
    cq_sb = consts.tile([128, NQ], F32)
    nc.scalar.dma_start(out=cq_sb, in_=cq[:])
    sq_sb = consts.tile([128, NQ], F32)
    nc.scalar.dma_start(out=sq_sb, in_=sq[:])
    ck_sb = consts.tile([128, KQ * 128], F32)
    nc.scalar.dma_start(out=ck_sb, in_=ck[:])
    sk_sb = consts.tile([128, KQ * 128], F32)
    nc.scalar.dma_start(out=sk_sb, in_=sk[:])
    eps_sb = consts.tile([128, 1], F32)
    nc.scalar.dma_start(out=eps_sb, in_=epsv[:])
    neps_sb = consts.tile([128, 1], F32)
    nc.scalar.dma_start(out=neps_sb, in_=nepsv[:])
    dlhs_sb = consts.tile([128, NPAIR, H], BF16)
    nc.scalar.dma_start(out=dlhs_sb, in_=dlhs[:])
    blhs_sb = consts.tile([H, NPAIR, 128], BF16)
    nc.scalar.dma_start(out=blhs_sb, in_=blhs[:])
    bout_sb = consts.tile([128, KQ], F32)
    nc.scalar.dma_start(out=bout_sb, in_=bout_t[:])
    wout_sb = consts.tile([128, KQ, DQ], BF16)
    nc.scalar.dma_start(out=wout_sb, in_=wout[:])

    # ---- phase 0: tokensT, kT, k-RoPE, v ----
    _ph0 = nc.named_scope("ph0")
    _ph0.__enter__()
    tokT_sb = kside.tile([128, KKV, 128], BF16)
    for ct in range(KKV):
        tp = pp.tile([128, 128], BF16, tag="pp")
        nc.tensor.transpose(tp, tok_sb[:, ct * 128:(ct + 1) * 128], id_sb[:])
        nc.scalar.copy(out=tokT_sb[:, ct, :], in_=tp)

    kT_sb = kside.tile([128, KQ, 128], F32)
    for m in range(KQ):
        kp = pp.tile([128, 128], F32, tag="pp")
        for kc in range(KKV):
            nc.tensor.matmul(kp, wk_sb[:, kc, m * 128:(m + 1) * 128],
                             tokT_sb[:, kc, :],
                             start=(kc == 0), stop=(kc == KKV - 1))
        nc.scalar.copy(out=kT_sb[:, m, :], in_=kp)

    kA_sb = kside.tile([128, KQ, 128], BF16)
    kB_sb = kside.tile([128, KQ, 128], BF16)
    t1 = ktmp.tile([128, KQ * 128], F32, tag="t1")
    t2 = ktmp.tile([128, KQ * 128], F32, tag="t2")
    t1s = ktmp.tile([128, KQ * 128], F32, tag="t1s")
    t2s = ktmp.tile([128, KQ * 128], F32, tag="t2s")
    nc.gpsimd.tensor_mul(t1, kT_sb[:], ck_sb[:])
    nc.gpsimd.tensor_mul(t2, kT_sb[:], sk_sb[:])
    _sigma_dma(nc, t1s, t1)
    _sigma_dma(nc, t2s, t2)
    # kA = k_rot = t1 + eps * sigma(t2);  kB = t2 - eps * sigma(t1)
    nc.vector.scalar_tensor_tensor(out=kA_sb[:], in0=t2s,
                                   scalar=eps_sb[:], in1=t1,
                                   op0=MULT, op1=ADD)
    nc.vector.scalar_tensor_tensor(out=kB_sb[:], in0=t1s,
                                   scalar=neps_sb[:], in1=t2,
                                   op0=MULT, op1=ADD)

    # v, natural [token, dim] layout, duplicated across partition halves:
    # vv[0:64, b, :] == vv[64:128, b, :] == v of batch b
    vv_sb = kside.tile([128, BL, DQ], BF16)
    for b in range(BL):
        for nn0 in range(0, DQ, 512):
            nsz = min(512, DQ - nn0)
            vp = pp.tile([128, 512], F32, tag="pp")
            for half in range(2):
                for kc in range(KKV):
                    nc.tensor.matmul(
                        vp[64 * half:64 * half + 64, :nsz],
                        tokT_sb[:, kc, b * 64:(b + 1) * 64],
                        wv_sb[:, kc, nn0:nn0 + nsz],
                        start=(kc == 0), stop=(kc == KKV - 1))
            nc.scalar.copy(out=vv_sb[:, b, nn0:nn0 + nsz], in_=vp[:, :nsz])

    _ph0.__exit__(None, None, None)

    # ---- main loop: software-pipelined across the 4 (batch, chunk) steps.
    # The PE issues strictly in program order, so each chunk's serial
    # attention chains (exp -> denom -> recip -> bcast -> normalize) are
    # covered by the next chunk's dense projection matmuls; without this the
    # PE array duty cycle drops and the HAM clock-gate rethrottles to 1.2GHz.
    chunks = [(b, c) for b in range(BL) for c in range(NCH)]
    st = {}

    def stage_qproj(i):
        b, c = chunks[i]
        p0 = c * CHUNK
        featb = featp.tile([128, KQ, CHUNK], BF16, tag="featb", name=f"fb{i}")
        nc.sync.dma_start(out=featb, in_=feat_bf[b, :, :, p0:p0 + CHUNK])
        qc_sb = qp.tile([128, KQ, CHUNK], BF16, tag="qc", name=f"qc{i}")
        qs_sb = qp.tile([128, KQ, CHUNK], BF16, tag="qs", name=f"qs{i}")
        for m in range(KQ):
            qps = pp.tile([128, CHUNK], F32, tag="pp", name=f"qp{i}_{m}")
            for kc in range(KQ):
                nc.tensor.matmul(qps,
                                 wq_sb[:, kc, m * 128:(m + 1) * 128],
                                 featb[:, kc, :],
                                 start=(kc == 0), stop=(kc == KQ - 1))
            nc.vector.tensor_mul(qc_sb[:, m, :], qps, cq_sb[:, p0:p0 + CHUNK])
            nc.vector.tensor_mul(qs_sb[:, m, :], qps, sq_sb[:, p0:p0 + CHUNK])
        st[i] = dict(qc=qc_sb, qs=qs_sb)

    def stage_qk(i):
        b, c = chunks[i]
        s = st[i]
        featc = featp.tile([128, KQ, CHUNK], F32, tag="featc", name=f"fc{i}")
        nc.gpsimd.dma_start(out=featc, in_=feat_l[b, :, :, c * CHUNK:(c + 1) * CHUNK])
        s["featc"] = featc
        qc_sb, qs_sb = s["qc"], s["qs"]
        e_sb = ep.tile([128, NPAIR, CHUNK], BF16, tag="e", name=f"e{i}")
        dps = dp.tile([H, CHUNK], F32, tag="den", name=f"d{i}")

        def qk1(j):
            sps = attn.tile([128, CHUNK], F32, tag="attn", name=f"s{i}_{j}")
            for lo in range(2):  # head 2j (partitions 0:64), 2j+1 (64:128)
                o = 64 * lo
                sl = slice(o, o + 64)
                nc.tensor.matmul(sps[sl, :],
                                 kA_sb[sl, j, b * 64:(b + 1) * 64],
                                 qc_sb[sl, j, :],
                                 start=True, stop=False)
                nc.tensor.matmul(sps[sl, :],
                                 kB_sb[sl, j, b * 64:(b + 1) * 64],
                                 qs_sb[sl, j, :],
                                 start=False, stop=True)
            nc.scalar.activation(out=e_sb[:, j, :], in_=sps, func=EXP)

        def denom(j):
            nc.tensor.matmul(dps, dlhs_sb[:, j, :],
                             e_sb[:, j, :],
                             start=(j == 0), stop=(j == NPAIR - 1))

        for j in range(NPAIR):
            qk1(j)
            if j >= 1:
                denom(j - 1)
        denom(NPAIR - 1)
        s["e"], s["dps"] = e_sb, dps

    def stage_recip(i):
        s = st[i]
        r32 = rp.tile([H, CHUNK], F32, tag="r32", name=f"r32_{i}")
        nc.vector.reciprocal_approx_fast(out=r32, in_=s["dps"])
        r_sb = rp.tile([H, CHUNK], BF16, tag="r", name=f"r{i}")
        nc.scalar.copy(out=r_sb, in_=r32)
        s["r"] = r_sb

    def stage_avbc(i):
        b, c = chunks[i]
        s = st[i]
        e_sb, r_sb = s["e"], s["r"]
        attnT_sb = atp.tile([128, NPAIR, CHUNK], BF16, tag="attnT",
                            name=f"at{i}")

        def av(j):
            aps = attn.tile([128, CHUNK], F32, tag="attn", name=f"a{i}_{j}")
            for lo in range(2):
                o = 64 * lo
                sl = slice(o, o + 64)
                nc.tensor.matmul(
                    aps[sl, :],
                    vv_sb[sl, b, (2 * j + lo) * 64:(2 * j + lo + 1) * 64],
                    e_sb[sl, j, :], start=True, stop=True)
            return aps

        def bcast(j):
            bps = attn.tile([128, CHUNK], F32, tag="attn", name=f"b{i}_{j}")
            nc.tensor.matmul(bps, blhs_sb[:, j, :],
                             r_sb[:], start=True, stop=True)
            # stage to SBUF (DVE may read only one PSUM operand)
            bcs = rp.tile([128, CHUNK], F32, tag="bcs", bufs=3,
                          name=f"bc{i}_{j}")
            nc.scalar.copy(out=bcs, in_=bps)
            return bcs

        av_t, bc_t = {}, {}
        for j in range(NPAIR):
            av_t[j] = av(j)
            bc_t[j] = bcast(j)
            if j >= 1:
                nc.vector.tensor_mul(attnT_sb[:, j - 1, :],
                                     av_t[j - 1], bc_t[j - 1])
        nc.vector.tensor_mul(attnT_sb[:, NPAIR - 1, :],
                             av_t[NPAIR - 1], bc_t[NPAIR - 1])
        s["attnT"] = attnT_sb

    def stage_oproj(i):
        b, c = chunks[i]
        p0 = c * CHUNK
        s = st[i]
        attnT_sb, featc = s["attnT"], s["featc"]
        o_sb = outp.tile([128, KQ, CHUNK], F32, tag="osb", name=f"o{i}")
        for m in range(KQ):
            ops = pp.tile([128, CHUNK], F32, tag="pp", name=f"op{i}_{m}")
            for kc in range(KQ):
                nc.tensor.matmul(ops,
                                 wout_sb[:, kc, m * 128:(m + 1) * 128],
                                 attnT_sb[:, kc, :],
                                 start=(kc == 0), stop=(kc == KQ - 1))
            nc.vector.scalar_tensor_tensor(out=o_sb[:, m, :], in0=ops,
                                           scalar=bout_sb[:, m:m + 1],
                                           in1=featc[:, m, :],
                                           op0=ADD, op1=ADD)
            if m in (1, 3):
                nc.sync.dma_start(out=out_l[b, :, m - 1:m + 1, p0:p0 + CHUNK],
                                  in_=o_sb[:, m - 1:m + 1, :])
        nc.sync.dma_start(out=out_l[b, :, 4:KQ, p0:p0 + CHUNK],
                          in_=o_sb[:, 4:KQ, :])

    def scoped(fn, tag, i):
        with nc.named_scope(f"{tag}{i}"):
            fn(i)

    scoped(stage_qproj, "qp", 0)
    scoped(stage_qk, "qk", 0)
    n = len(chunks)
    for i in range(n):
        scoped(stage_recip, "rc", i)
        if i + 1 < n:
            scoped(stage_qproj, "qp", i + 1)
        scoped(stage_avbc, "av", i)
        if i + 1 < n:
            scoped(stage_qk, "qk", i + 1)
        scoped(stage_oproj, "op", i)


_NC_CACHE = {}


def _get_nc():
    if "nc" not in _NC_CACHE:
        _NC_CACHE["nc"] = build(debug=False)
    return _NC_CACHE["nc"]


def _prep_in_maps(feat, tokens, Wq, Wkv, Wout, bout):
    feat = np.ascontiguousarray(feat, dtype=np.float32).reshape(B, DQ, NQ)
    tokens = np.ascontiguousarray(tokens, dtype=np.float32)
    shared = dict(
        wq=np.ascontiguousarray(
            Wq.reshape(KQ, 128, DQ).transpose(1, 0, 2), dtype=NPBF),
        wk=np.ascontiguousarray(
            Wkv[:, :DQ].reshape(KKV, 128, DQ).transpose(1, 0, 2), dtype=NPBF),
        wv=np.ascontiguousarray(
            Wkv[:, DQ:].reshape(KKV, 128, DQ).transpose(1, 0, 2), dtype=NPBF),
        wout=np.ascontiguousarray(
            Wout.reshape(KQ, 128, DQ).transpose(1, 0, 2), dtype=NPBF),
        bout_t=np.ascontiguousarray(bout.reshape(KQ, 128).T, dtype=np.float32),
        **_consts(),
    )
    in_maps = []
    for cid in range(NCORES):
        sl = slice(BL * cid, BL * (cid + 1))
        fl = np.ascontiguousarray(
            feat[sl].reshape(BL, KQ, 128, NQ).transpose(0, 2, 1, 3))
        tl = np.ascontiguousarray(tokens[sl].reshape(BL * NT, DKV), dtype=NPBF)
        in_maps.append(dict(feat_l=fl, feat_bf=fl.astype(NPBF), tok_l=tl,
                            **shared))
    return in_maps


def _install_ntff_hook():
    """The container's antenv lacks axon_hooks; register the NTFF profile
    hook from trn_agent_boot ourselves so trace=True yields HW exec times."""
    import types

    import antenv
    from trn_agent_boot.trn_boot import _ntff_profile_via_ctypes

    mod = types.ModuleType("antenv.axon_hooks")
    state = {"hook": None}
    mod.set_axon_ntff_profile_hook = lambda h: state.__setitem__("hook", h)
    mod.get_axon_ntff_profile_hook = lambda: state["hook"]
    sys.modules["antenv.axon_hooks"] = mod
    antenv.axon_hooks = mod
    mod.set_axon_ntff_profile_hook(
        _ntff_profile_via_ctypes("/opt/axon/libaxon_pjrt.so"))
    # the S3 artifact upload has no credentials here; make it a no-op
    import concourse.bass_utils as bu
    bu.upload_artifacts = lambda tmpdir: f"local:{tmpdir}"


def run(inputs, trace=False, trace_cores=None):
    nc = _get_nc()
    if trace:
        try:
            _install_ntff_hook()
        except Exception as e:  # profiling is best-effort
            print(f"ntff hook install failed: {e}", file=sys.stderr)
            trace = False
    in_maps = _prep_in_maps(**inputs)
    res = run_bass_kernel_spmd(nc, in_maps, core_ids=list(range(NCORES)),
                               trace=trace, trace_cores=trace_cores)
    outs = []
    for r in res.results:
        ol = r["out_l"]  # [BL, 128, KQ, NQ]
        outs.append(ol.transpose(0, 2, 1, 3).reshape(BL, DQ, T, HP, WP))
    return np.ascontiguousarray(np.concatenate(outs, axis=0)), res


def kernel(**inputs):
    return run(inputs, trace=False)[0]

